# revision 9
# baseline (speedup 1.0000x reference)
"""DGCNN edge-conv kernel for Trainium2 (8 NeuronCores, data-parallel over clouds).

Reference computation (per cloud of P=2048 points, D=64 feats):
    idx = knn(x, K=16)  (self excluded, by squared L2 distance)
    e   = [x_i, x_j - x_i]  -> h = e @ W.T + b -> out = relu(max_k h)

Algebraic rewrite used here: with W = [W1 | W2],
    h[p,k] = x_p @ (W1-W2).T + x_{n(p,k)} @ W2.T + b
so with a_p = x_p @ (W1-W2).T + b (constant over k) and c_j = x_j @ W2.T:
    out[p] = relu(a_p + max_k c_{n(p,k)})

Per-core program (2 clouds/core):
  - build B = [x^T ; -|x_j|^2] (65 x 2048) and A2 = [2 x^T ; 1] via PE transpose
  - s = A2_chunk^T @ B = 2 x_p.x_j - |x_j|^2  (PSUM, 128 x 2048 per chunk);
    ranking by s == ranking by -d2 (row-constant |x_p|^2 dropped)
  - top-16 per row: max8 / max_index / match_replace / max8 / max_index (DVE)
  - c rows gathered from DRAM via indirect DMA, reduced with strided reduce_max
  - out = relu(kmax + a) stored per chunk
"""

import numpy as np

import concourse.bass as bass
import concourse.bacc as bacc
import concourse.mybir as mybir
from concourse.bass_utils import run_bass_kernel_spmd
from concourse.masks import make_identity
from concourse.tile import TileContext

B, P, D, K, OUT = 16, 2048, 64, 16, 128
USE_DMA_GATHER = False
N_CORES = 8
CLOUDS_PER_CORE = B // N_CORES          # 2
ROWS_PER_CORE = CLOUDS_PER_CORE * P     # 4096
N_CHUNKS = P // 128                     # 16 chunks of 128 points per cloud
NEG_BIG = -1.0e30
F32 = mybir.dt.float32
U32 = mybir.dt.uint32
AF = mybir.ActivationFunctionType


def build_program(reps: int = 1) -> bass.Bass:
    nc = bacc.Bacc(None, target_bir_lowering=False)
    x_in = nc.declare_dram_parameter("x", [ROWS_PER_CORE, D], F32, isOutput=False)
    w_in = nc.declare_dram_parameter("W", [OUT, 2 * D], F32, isOutput=False)
    b_in = nc.declare_dram_parameter("b", [OUT], F32, isOutput=False)
    out_ext = nc.declare_dram_parameter("out", [ROWS_PER_CORE, OUT], F32, isOutput=True)
    # per-cloud c tables (indirect DMA sources must sit at offset 0)
    c_dram = [nc.dram_tensor(f"c_tab{ci}", [P, OUT], F32) for ci in range(CLOUDS_PER_CORE)]
    I16 = mybir.dt.int16
    U16 = mybir.dt.uint16
    # double-buffered DRAM scratch for the wrapped dma_gather index layout
    idx_scr = [nc.dram_tensor(f"idx_scr{j}", [P], I16) for j in range(2)]

    with TileContext(nc) as tc:
        with tc.tile_pool(name="const", bufs=1) as constp:
            ident = constp.tile([128, 128], F32)
            make_identity(nc, ident)
            negid = constp.tile([128, 128], F32)
            nc.vector.tensor_scalar(negid, ident, NEG_BIG, scalar2=None,
                                    op0=mybir.AluOpType.mult)
            w_sb = constp.tile([OUT, 2 * D], F32)
            nc.sync.dma_start(out=w_sb, in_=w_in[:])
            w12 = constp.tile([OUT, D], F32)
            nc.vector.tensor_sub(w12, w_sb[:, 0:D], w_sb[:, D:2 * D])
            b_sb = constp.tile([1, OUT], F32)
            nc.sync.dma_start(out=b_sb, in_=b_in[:].unsqueeze(0))
            ones_row = constp.tile([1, 128], F32)
            nc.vector.memset(ones_row, 1.0)
            w2t = constp.tile([D, OUT], F32)
            w12t = constp.tile([D, OUT], F32)

            with tc.tile_pool(name="wps", bufs=1, space="PSUM") as wps:
                w2t_ps = wps.tile([D, OUT], F32)
                nc.tensor.transpose(out=w2t_ps, in_=w_sb[:, D:2 * D], identity=ident)
                nc.scalar.activation(out=w2t, in_=w2t_ps, func=AF.Copy)
                w12t_ps = wps.tile([D, OUT], F32)
                nc.tensor.transpose(out=w12t_ps, in_=w12, identity=ident)
                nc.scalar.activation(out=w12t, in_=w12t_ps, func=AF.Copy)

            with tc.tile_pool(name="big", bufs=2) as bigp, \
                 tc.tile_pool(name="ph0ps", bufs=1, space="PSUM") as ph0ps, \
                 tc.tile_pool(name="sps", bufs=1, space="PSUM") as sps, \
                 tc.tile_pool(name="work", bufs=2) as workp:
                for ci in [c for _ in range(reps) for c in range(CLOUDS_PER_CORE)]:
                    row0 = ci * P
                    # ---------------- phase 0: transposed features + c/a ----------
                    # Bm = [x^T ; -sq], A2 = [2 x^T ; 1]; each 512-col slice of
                    # Bm/A2 has exactly one producer (one ACT copy) so matmuls
                    # don't accumulate too many sem waits.
                    Bm = bigp.tile([D + 1, P], F32, tag="Bm")
                    A2 = bigp.tile([D + 1, P], F32, tag="A2")
                    a_all = bigp.tile([128, P], F32, tag="a_all")
                    for t4 in range(N_CHUNKS // 4):
                        tp = ph0ps.tile([D + 1, 512], F32, tag="tp")
                        tp2 = ph0ps.tile([D + 1, 512], F32, tag="tp2")
                        for u in range(4):
                            t = t4 * 4 + u
                            ucols = bass.ts(u, 128)
                            aug = workp.tile([128, 72], F32, tag="aug")
                            aug2 = workp.tile([128, 72], F32, tag="aug2")
                            nc.sync.dma_start(
                                out=aug[:, 0:D],
                                in_=x_in[row0 + t * 128:row0 + (t + 1) * 128, :])
                            sqd = workp.tile([128, D], F32, tag="sqd")
                            sq = workp.tile([128, 8], F32, tag="sq")
                            nc.scalar.activation(out=sqd, in_=aug[:, 0:D],
                                                 func=AF.Square, accum_out=sq[:, 0:1])
                            nc.vector.tensor_scalar(aug[:, D:D + 1], sq[:, 0:1], -1.0,
                                                    scalar2=None,
                                                    op0=mybir.AluOpType.mult)
                            nc.scalar.activation(out=aug2[:, 0:D], in_=aug[:, 0:D],
                                                 func=AF.Copy, scale=2.0)
                            nc.gpsimd.memset(aug2[:, D:D + 1], 1.0)
                            nc.tensor.transpose(out=tp[:, ucols], in_=aug[:, 0:D + 1],
                                                identity=ident)
                            nc.tensor.transpose(out=tp2[:, ucols], in_=aug2[:, 0:D + 1],
                                                identity=ident)
                        scols = bass.ts(t4, 512)
                        nc.scalar.activation(out=Bm[:, scols], in_=tp, func=AF.Copy)
                        nc.scalar.activation(out=A2[:, scols], in_=tp2, func=AF.Copy)
                    for t in range(N_CHUNKS):
                        cols = bass.ts(t, 128)
                        cps = ph0ps.tile([128, OUT], F32, tag="cps")
                        nc.tensor.matmul(out=cps, lhsT=Bm[0:D, cols], rhs=w2t,
                                         start=True, stop=True)
                        cst = workp.tile([128, OUT], F32, tag="cst")
                        nc.scalar.activation(out=cst, in_=cps, func=AF.Copy)
                        nc.sync.dma_start(out=c_dram[ci][t * 128:(t + 1) * 128, :], in_=cst)
                        aps = ph0ps.tile([128, OUT], F32, tag="aps")
                        nc.tensor.matmul(out=aps, lhsT=Bm[0:D, cols], rhs=w12t,
                                         start=True, stop=False)
                        nc.tensor.matmul(out=aps, lhsT=ones_row, rhs=b_sb,
                                         start=False, stop=True)
                        nc.scalar.activation(out=a_all[:, cols], in_=aps, func=AF.Copy)

                    # ---------------- phase 1: distances + top-16 + aggregate ------
                    for t in range(N_CHUNKS):
                        cols = bass.ts(t, 128)
                        s_ps = sps.tile([128, P], F32, tag="s")
                        for n4 in range(P // 512):
                            nc.tensor.matmul(out=s_ps[:, bass.ts(n4, 512)],
                                             lhsT=A2[:, cols],
                                             rhs=Bm[:, bass.ts(n4, 512)],
                                             start=True, stop=True)
                        # self-exclusion on the diagonal block
                        nc.vector.tensor_add(s_ps[:, cols], s_ps[:, cols], negid)
                        v1 = workp.tile([128, 8], F32, tag="v1")
                        v2 = workp.tile([128, 8], F32, tag="v2")
                        idx_dt = U16 if USE_DMA_GATHER else U32
                        idx = workp.tile([128, 2 * 8], idx_dt, tag="idx")
                        nc.vector.max(out=v1, in_=s_ps)
                        nc.vector.max_index(out=idx[:, 0:8], in_max=v1, in_values=s_ps)
                        sp = workp.tile([128, P], F32, tag="sp")
                        nc.vector.match_replace(out=sp, in_to_replace=v1, in_values=s_ps,
                                                imm_value=NEG_BIG)
                        nc.vector.max(out=v2, in_=sp)
                        nc.vector.max_index(out=idx[:, 8:16], in_max=v2, in_values=sp)
                        gat = workp.tile([128, K, OUT], F32, tag="gat")
                        if USE_DMA_GATHER:
                            # One HW-descriptor-generated gather for all 2048
                            # rows. Index list must be "wrapped": flat[i] =
                            # idxs[i%16, i//16] with flat[c*128+p] = n(p, c);
                            # that layout is T[q, 8c+d] = idx[16d+q, c], built
                            # via PE transpose + an 8-way strided DRAM shuffle.
                            idxf = workp.tile([128, 2 * 8], F32, tag="idxf")
                            nc.vector.tensor_copy(out=idxf, in_=idx)
                            tpi = ph0ps.tile([2 * 8, 128], F32, tag="cps")
                            nc.tensor.transpose(out=tpi, in_=idxf, identity=ident)
                            idxT = workp.tile([2 * 8, 128], I16, tag="idxT")
                            nc.vector.tensor_copy(out=idxT, in_=tpi)
                            scr = idx_scr[t % 2]
                            for dd in range(8):
                                ov = scr[:].rearrange("(r c d) -> c r d",
                                                      r=16, c=16, d=8)[:, :, dd]
                                nc.sync.dma_start(out=ov,
                                                  in_=idxT[:, 16 * dd:16 * dd + 16])
                            Tt = workp.tile([128, 128], I16, tag="Tt")
                            nc.vector.memset(Tt, 0)
                            nc.sync.dma_start(
                                out=Tt[0:16, :],
                                in_=scr[:].rearrange("(r s) -> r s", r=16))
                            nc.gpsimd.dma_gather(
                                out_ap=gat[:], in_ap=c_dram[ci][:], idxs_ap=Tt[:],
                                num_idxs=P, num_idxs_reg=P, elem_size=OUT)
                        else:
                            # fallback: one offset per partition per transfer
                            for k in range(K):
                                nc.gpsimd.indirect_dma_start(
                                    out=gat[:, k, :],
                                    out_offset=None,
                                    in_=c_dram[ci][:],
                                    in_offset=bass.IndirectOffsetOnAxis(
                                        ap=idx[:, k:k + 1], axis=0),
                                )
                        km = workp.tile([128, OUT], F32, tag="km")
                        nc.vector.reduce_max(out=km, in_=gat[:].transpose([0, 2, 1]),
                                             axis=mybir.AxisListType.X)
                        oadd = workp.tile([128, OUT], F32, tag="oadd")
                        nc.vector.tensor_add(oadd, km, a_all[:, cols])
                        ost = workp.tile([128, OUT], F32, tag="ost")
                        nc.scalar.activation(out=ost, in_=oadd, func=AF.Relu)
                        nc.sync.dma_start(
                            out=out_ext[row0 + t * 128:row0 + (t + 1) * 128, :], in_=ost)
    nc.compile()
    return nc


_program_cache = None


def _get_program() -> bass.Bass:
    global _program_cache
    if _program_cache is None:
        _program_cache = build_program()
    return _program_cache


def kernel(**inputs: np.ndarray) -> np.ndarray:
    x = np.ascontiguousarray(np.asarray(inputs["x"], dtype=np.float32))
    W = np.ascontiguousarray(np.asarray(inputs["W"], dtype=np.float32))
    b = np.ascontiguousarray(np.asarray(inputs["b"], dtype=np.float32))

    nc = _get_program()
    in_maps = [
        {"x": x[i * ROWS_PER_CORE:(i + 1) * ROWS_PER_CORE], "W": W, "b": b}
        for i in range(N_CORES)
    ]
    res = run_bass_kernel_spmd(nc, in_maps, list(range(N_CORES)))
    return np.concatenate([res.results[i]["out"] for i in range(N_CORES)], axis=0)


# revision 10
# speedup vs baseline: 1.0006x; 1.0006x over previous
"""DGCNN edge-conv kernel for Trainium2 (8 NeuronCores, data-parallel over clouds).

Reference computation (per cloud of P=2048 points, D=64 feats):
    idx = knn(x, K=16)  (self excluded, by squared L2 distance)
    e   = [x_i, x_j - x_i]  -> h = e @ W.T + b -> out = relu(max_k h)

Algebraic rewrite used here: with W = [W1 | W2],
    h[p,k] = x_p @ (W1-W2).T + x_{n(p,k)} @ W2.T + b
so with a_p = x_p @ (W1-W2).T + b (constant over k) and c_j = x_j @ W2.T:
    out[p] = relu(a_p + max_k c_{n(p,k)})

Per-core program (2 clouds/core):
  - build B = [x^T ; -|x_j|^2] (65 x 2048) and A2 = [2 x^T ; 1] via PE transpose
  - s = A2_chunk^T @ B = 2 x_p.x_j - |x_j|^2  (PSUM, 128 x 2048 per chunk);
    ranking by s == ranking by -d2 (row-constant |x_p|^2 dropped)
  - top-16 per row: max8 / max_index / match_replace / max8 / max_index (DVE)
  - c rows gathered from DRAM via indirect DMA, reduced with strided reduce_max
  - out = relu(kmax + a) stored per chunk
"""

import numpy as np

import concourse.bass as bass
import concourse.bacc as bacc
import concourse.mybir as mybir
from concourse.bass_utils import run_bass_kernel_spmd
from concourse.masks import make_identity
from concourse.tile import TileContext

B, P, D, K, OUT = 16, 2048, 64, 16, 128
USE_DMA_GATHER = False
N_CORES = 8
CLOUDS_PER_CORE = B // N_CORES          # 2
ROWS_PER_CORE = CLOUDS_PER_CORE * P     # 4096
N_CHUNKS = P // 128                     # 16 chunks of 128 points per cloud
NEG_BIG = -1.0e30
F32 = mybir.dt.float32
U32 = mybir.dt.uint32
AF = mybir.ActivationFunctionType


def build_program(reps: int = 1) -> bass.Bass:
    nc = bacc.Bacc(None, target_bir_lowering=False)
    x_in = nc.declare_dram_parameter("x", [ROWS_PER_CORE, D], F32, isOutput=False)
    w_in = nc.declare_dram_parameter("W", [OUT, 2 * D], F32, isOutput=False)
    b_in = nc.declare_dram_parameter("b", [OUT], F32, isOutput=False)
    out_ext = nc.declare_dram_parameter("out", [ROWS_PER_CORE, OUT], F32, isOutput=True)
    # per-cloud c tables (indirect DMA sources must sit at offset 0)
    c_dram = [nc.dram_tensor(f"c_tab{ci}", [P, OUT], F32) for ci in range(CLOUDS_PER_CORE)]
    I16 = mybir.dt.int16
    U16 = mybir.dt.uint16
    # double-buffered DRAM scratch for the wrapped dma_gather index layout
    idx_scr = [nc.dram_tensor(f"idx_scr{j}", [P], I16) for j in range(2)]

    with TileContext(nc) as tc:
        with tc.tile_pool(name="const", bufs=1) as constp:
            ident = constp.tile([128, 128], F32)
            make_identity(nc, ident)
            negid = constp.tile([128, 128], F32)
            nc.vector.tensor_scalar(negid, ident, NEG_BIG, scalar2=None,
                                    op0=mybir.AluOpType.mult)
            w_sb = constp.tile([OUT, 2 * D], F32)
            nc.sync.dma_start(out=w_sb, in_=w_in[:])
            w12 = constp.tile([OUT, D], F32)
            nc.vector.tensor_sub(w12, w_sb[:, 0:D], w_sb[:, D:2 * D])
            b_sb = constp.tile([1, OUT], F32)
            nc.sync.dma_start(out=b_sb, in_=b_in[:].unsqueeze(0))
            ones_row = constp.tile([1, 128], F32)
            nc.vector.memset(ones_row, 1.0)
            w2t = constp.tile([D, OUT], F32)
            w12t = constp.tile([D, OUT], F32)

            with tc.tile_pool(name="wps", bufs=1, space="PSUM") as wps:
                w2t_ps = wps.tile([D, OUT], F32)
                nc.tensor.transpose(out=w2t_ps, in_=w_sb[:, D:2 * D], identity=ident)
                nc.scalar.activation(out=w2t, in_=w2t_ps, func=AF.Copy)
                w12t_ps = wps.tile([D, OUT], F32)
                nc.tensor.transpose(out=w12t_ps, in_=w12, identity=ident)
                nc.scalar.activation(out=w12t, in_=w12t_ps, func=AF.Copy)

            with tc.tile_pool(name="big", bufs=2) as bigp, \
                 tc.tile_pool(name="ph0ps", bufs=1, space="PSUM") as ph0ps, \
                 tc.tile_pool(name="sps", bufs=1, space="PSUM") as sps, \
                 tc.tile_pool(name="work", bufs=2) as workp:
                for ci in [c for _ in range(reps) for c in range(CLOUDS_PER_CORE)]:
                    row0 = ci * P
                    # ---------------- phase 0: transposed features + c/a ----------
                    # Bm = [x^T ; -sq], A2 = [2 x^T ; 1]; each 512-col slice of
                    # Bm/A2 has exactly one producer (one ACT copy) so matmuls
                    # don't accumulate too many sem waits.
                    Bm = bigp.tile([D + 1, P], F32, tag="Bm")
                    A2 = bigp.tile([D + 1, P], F32, tag="A2")
                    a_all = bigp.tile([128, P], F32, tag="a_all")
                    for t4 in range(N_CHUNKS // 4):
                        tp = ph0ps.tile([D + 1, 512], F32, tag="tp")
                        tp2 = ph0ps.tile([D + 1, 512], F32, tag="tp2")
                        for u in range(4):
                            t = t4 * 4 + u
                            ucols = bass.ts(u, 128)
                            aug = workp.tile([128, 72], F32, tag="aug")
                            aug2 = workp.tile([128, 72], F32, tag="aug2")
                            nc.sync.dma_start(
                                out=aug[:, 0:D],
                                in_=x_in[row0 + t * 128:row0 + (t + 1) * 128, :])
                            sqd = workp.tile([128, D], F32, tag="sqd")
                            sq = workp.tile([128, 8], F32, tag="sq")
                            nc.scalar.activation(out=sqd, in_=aug[:, 0:D],
                                                 func=AF.Square, accum_out=sq[:, 0:1])
                            nc.vector.tensor_scalar(aug[:, D:D + 1], sq[:, 0:1], -1.0,
                                                    scalar2=None,
                                                    op0=mybir.AluOpType.mult)
                            nc.scalar.activation(out=aug2[:, 0:D], in_=aug[:, 0:D],
                                                 func=AF.Copy, scale=2.0)
                            nc.vector.memset(aug2[:, D:D + 1], 1.0)
                            nc.tensor.transpose(out=tp[:, ucols], in_=aug[:, 0:D + 1],
                                                identity=ident)
                            nc.tensor.transpose(out=tp2[:, ucols], in_=aug2[:, 0:D + 1],
                                                identity=ident)
                        scols = bass.ts(t4, 512)
                        nc.scalar.activation(out=Bm[:, scols], in_=tp, func=AF.Copy)
                        nc.scalar.activation(out=A2[:, scols], in_=tp2, func=AF.Copy)
                    for t in range(N_CHUNKS):
                        cols = bass.ts(t, 128)
                        cps = ph0ps.tile([128, OUT], F32, tag="cps")
                        nc.tensor.matmul(out=cps, lhsT=Bm[0:D, cols], rhs=w2t,
                                         start=True, stop=True)
                        cst = workp.tile([128, OUT], F32, tag="cst")
                        nc.scalar.activation(out=cst, in_=cps, func=AF.Copy)
                        nc.sync.dma_start(out=c_dram[ci][t * 128:(t + 1) * 128, :], in_=cst)
                        aps = ph0ps.tile([128, OUT], F32, tag="aps")
                        nc.tensor.matmul(out=aps, lhsT=Bm[0:D, cols], rhs=w12t,
                                         start=True, stop=False)
                        nc.tensor.matmul(out=aps, lhsT=ones_row, rhs=b_sb,
                                         start=False, stop=True)
                        nc.scalar.activation(out=a_all[:, cols], in_=aps, func=AF.Copy)

                    # ---------------- phase 1: distances + top-16 + aggregate ------
                    for t in range(N_CHUNKS):
                        cols = bass.ts(t, 128)
                        s_ps = sps.tile([128, P], F32, tag="s")
                        for n4 in range(P // 512):
                            nc.tensor.matmul(out=s_ps[:, bass.ts(n4, 512)],
                                             lhsT=A2[:, cols],
                                             rhs=Bm[:, bass.ts(n4, 512)],
                                             start=True, stop=True)
                        # self-exclusion on the diagonal block
                        nc.vector.tensor_add(s_ps[:, cols], s_ps[:, cols], negid)
                        v1 = workp.tile([128, 8], F32, tag="v1")
                        v2 = workp.tile([128, 8], F32, tag="v2")
                        idx_dt = U16 if USE_DMA_GATHER else U32
                        idx = workp.tile([128, 2 * 8], idx_dt, tag="idx")
                        nc.vector.max(out=v1, in_=s_ps)
                        nc.vector.max_index(out=idx[:, 0:8], in_max=v1, in_values=s_ps)
                        sp = workp.tile([128, P], F32, tag="sp")
                        nc.vector.match_replace(out=sp, in_to_replace=v1, in_values=s_ps,
                                                imm_value=NEG_BIG)
                        nc.vector.max(out=v2, in_=sp)
                        nc.vector.max_index(out=idx[:, 8:16], in_max=v2, in_values=sp)
                        gat = workp.tile([128, K, OUT], F32, tag="gat")
                        if USE_DMA_GATHER:
                            # One HW-descriptor-generated gather for all 2048
                            # rows. Index list must be "wrapped": flat[i] =
                            # idxs[i%16, i//16] with flat[c*128+p] = n(p, c);
                            # that layout is T[q, 8c+d] = idx[16d+q, c], built
                            # via PE transpose + an 8-way strided DRAM shuffle.
                            idxf = workp.tile([128, 2 * 8], F32, tag="idxf")
                            nc.vector.tensor_copy(out=idxf, in_=idx)
                            tpi = ph0ps.tile([2 * 8, 128], F32, tag="cps")
                            nc.tensor.transpose(out=tpi, in_=idxf, identity=ident)
                            idxT = workp.tile([2 * 8, 128], I16, tag="idxT")
                            nc.vector.tensor_copy(out=idxT, in_=tpi)
                            scr = idx_scr[t % 2]
                            for dd in range(8):
                                ov = scr[:].rearrange("(r c d) -> c r d",
                                                      r=16, c=16, d=8)[:, :, dd]
                                nc.sync.dma_start(out=ov,
                                                  in_=idxT[:, 16 * dd:16 * dd + 16])
                            Tt = workp.tile([128, 128], I16, tag="Tt")
                            nc.vector.memset(Tt, 0)
                            nc.sync.dma_start(
                                out=Tt[0:16, :],
                                in_=scr[:].rearrange("(r s) -> r s", r=16))
                            nc.gpsimd.dma_gather(
                                out_ap=gat[:], in_ap=c_dram[ci][:], idxs_ap=Tt[:],
                                num_idxs=P, num_idxs_reg=P, elem_size=OUT)
                        else:
                            # fallback: one offset per partition per transfer
                            for k in range(K):
                                nc.gpsimd.indirect_dma_start(
                                    out=gat[:, k, :],
                                    out_offset=None,
                                    in_=c_dram[ci][:],
                                    in_offset=bass.IndirectOffsetOnAxis(
                                        ap=idx[:, k:k + 1], axis=0),
                                )
                        km = workp.tile([128, OUT], F32, tag="km")
                        nc.vector.reduce_max(out=km, in_=gat[:].transpose([0, 2, 1]),
                                             axis=mybir.AxisListType.X)
                        oadd = workp.tile([128, OUT], F32, tag="oadd")
                        nc.vector.tensor_add(oadd, km, a_all[:, cols])
                        ost = workp.tile([128, OUT], F32, tag="ost")
                        nc.scalar.activation(out=ost, in_=oadd, func=AF.Relu)
                        nc.sync.dma_start(
                            out=out_ext[row0 + t * 128:row0 + (t + 1) * 128, :], in_=ost)
    nc.compile()
    return nc


_program_cache = None


def _get_program() -> bass.Bass:
    global _program_cache
    if _program_cache is None:
        _program_cache = build_program()
    return _program_cache


def kernel(**inputs: np.ndarray) -> np.ndarray:
    x = np.ascontiguousarray(np.asarray(inputs["x"], dtype=np.float32))
    W = np.ascontiguousarray(np.asarray(inputs["W"], dtype=np.float32))
    b = np.ascontiguousarray(np.asarray(inputs["b"], dtype=np.float32))

    nc = _get_program()
    in_maps = [
        {"x": x[i * ROWS_PER_CORE:(i + 1) * ROWS_PER_CORE], "W": W, "b": b}
        for i in range(N_CORES)
    ]
    res = run_bass_kernel_spmd(nc, in_maps, list(range(N_CORES)))
    return np.concatenate([res.results[i]["out"] for i in range(N_CORES)], axis=0)
